# revision 11
# baseline (speedup 1.0000x reference)
"""Trainium2 Bass kernel for nn_CropPrompter.

Fused resize+crop bilinear sampling of video clips:
  x[8,3,16,512,512] --(per-clip crop geometry from cam_views/resize/offsets)-->
  out[8,3,16,224,224]

Strategy (pure data parallel, 1 clip per NeuronCore, 8 cores):
  * Host computes, in float32 (bit-matching the reference math), the source
    coordinates and bilinear weights per clip, and packs them as two sparse
    interpolation matrices RyT [256,256] / RxT [256,256] (2 nonzeros per
    output column).  Because resize >= H=512 and offsets < 32, every clip's
    source window provably lies in the fixed [0,256) x [0,256) corner of each
    frame, so the device program is fully static and identical across cores —
    only the input *data* differs per core (SPMD).
  * Device, per frame: out = Ry @ win @ Rx^T via two TensorE matmul pairs
    (K split 2x128), staged as
      A^T[w,i]  = sum_h win[h,w] * RyT[h,i]   (lhsT=win tile, rhs=RyT)
      out[i,j]  = sum_w A^T[w,i] * RxT[w,j]   (lhsT=A^T tile, rhs=RxT)
    in float32r (PE full rate; fp32 matmul is 4x slower), with the moving
    free dim zero-padded to 256 (fp32r full rate needs >=256).
  * DMA reads only the [0:256, 0:256] window (12.6 MB/clip instead of 50 MB)
    in >=1 MiB transfers; output written back in 2 transfers per channel.
"""

import numpy as np

CROP = 224
H = 512
RESIZE_MAX = 1024
WIN = 256  # static source window (rows and cols) — proven upper bound
PAD = 256  # zero-padded output free dim so fp32r streams at 1 cycle/row

_PROGRAM = None
TRACE = False
LAST_RESULTS = None


def _coords(off, rb):
    """Replicates reference._coords in numpy float32, op-for-op."""
    i = np.arange(CROP, dtype=np.float32)
    src = (np.float32(off) + i + np.float32(0.5)) * (np.float32(H) / np.float32(rb)) - np.float32(0.5)
    src = np.maximum(src, np.float32(0.0))
    i0 = np.clip(np.floor(src).astype(np.int32), 0, H - 1)
    i1 = np.minimum(i0 + 1, H - 1)
    w = src - i0.astype(np.float32)
    return i0, i1, w


def _interp_matrix(off, rb):
    """[WIN, PAD] float32 M with M[src_row, out_idx] = bilinear weight."""
    i0, i1, w = _coords(off, rb)
    assert i0.min() >= 0 and i1.max() < WIN, (i0.min(), i1.max())
    m = np.zeros((WIN, PAD), dtype=np.float32)
    idx = np.arange(CROP)
    np.add.at(m, (i0, idx), np.float32(1.0) - w)
    np.add.at(m, (i1, idx), w)
    return m


def _split_multi_waits(nc):
    """Walrus (kernel-dev pipeline) allows only one semaphore wait per
    instruction; hoist extra waits onto standalone EventSemaphore
    instructions inserted just before, on the same engine."""
    from concourse import mybir

    n = 0
    for fn in nc.m.functions:
        for bb in fn.blocks:
            out = []
            changed = False
            for inst in bb.instructions:
                si = getattr(inst, "sync_info", None)
                waits = list(si.on_wait) if si is not None and si.on_wait else []
                if len(waits) > 1:
                    for k, w in enumerate(waits[:-1]):
                        out.append(
                            mybir.InstEventSemaphore(
                                name=f"{inst.name}-w{k}",
                                ins=[],
                                outs=[],
                                engine=inst.engine,
                                sync_info=mybir.SyncInfo(on_wait=[w], on_update=[]),
                            )
                        )
                        n += 1
                    inst.sync_info = mybir.SyncInfo(
                        on_wait=[waits[-1]], on_update=list(si.on_update or [])
                    )
                    changed = True
                out.append(inst)
            if changed:
                bb.instructions = out
    return n


def _build_program():
    from concourse import bass, mybir, tile

    f32 = mybir.dt.float32
    f32r = mybir.dt.float32r

    nc = bass.Bass()
    xc = nc.dram_tensor("xc", [3, 16, H, H], f32r, kind="ExternalInput")
    ry = nc.dram_tensor("ry", [128, 2, PAD], f32r, kind="ExternalInput")
    rx = nc.dram_tensor("rx", [128, 2, PAD], f32r, kind="ExternalInput")
    out = nc.dram_tensor("out", [3, 16, CROP, CROP], f32, kind="ExternalOutput")

    # The fused fp32r matmul (self-loading weights) lowers to a single S3_LW
    # instruction with room for only ONE semaphore wait.  All PSUM->SBUF
    # copies therefore go through VectorE alone (one engine => one sem covers
    # all RAW/WAR deps), and LDWEIGHTS "wait carriers" — which read SBUF but
    # write nothing, so they never pick up PSUM-drain waits of their own —
    # absorb each cross-engine semaphore onto the PE clock before any real
    # (group-starting) matmul needs it.
    bf16 = mybir.dt.bfloat16

    def pe_wait_carrier(ap):
        # tiny LDWEIGHTS of a 4-column bf16 view; the stationary register is
        # rewritten by the next self-loading matmul, so this is side-effect
        # free on PE state
        nc.tensor.ldweights(ap.bitcast(bf16))

    with tile.TileContext(nc) as tc:
        with (
            tc.tile_pool(name="const", bufs=1) as constp,
            tc.tile_pool(name="xin", bufs=2) as xinp,
            tc.tile_pool(name="atp", bufs=3) as atp,
            tc.tile_pool(name="otp", bufs=2) as otp,
            tc.tile_pool(name="psa", bufs=2, space="PSUM") as psap,
            tc.tile_pool(name="pso", bufs=2, space="PSUM") as psop,
        ):
            ryt = constp.tile([128, 2, PAD], f32r)
            rxt = constp.tile([128, 2, PAD], f32r)
            nc.sync.dma_start(out=ryt[:], in_=ry[:])
            nc.sync.dma_start(out=rxt[:], in_=rx[:])

            # consume the const-DMA semaphores (1 wait each)
            pe_wait_carrier(ryt[:, 0, 0:2])
            pe_wait_carrier(rxt[:, 0, 0:2])

            for c in range(3):
                # window tile: [p, t, q, w] with source row h = q*128+p
                xw = xinp.tile([128, 16, 2, WIN], f32r)
                for q in range(2):
                    nc.sync.dma_start(
                        out=xw[:, :, q, :],
                        in_=xc[c, :, q * 128 : (q + 1) * 128, 0:WIN].rearrange(
                            "t p w -> p t w"
                        ),
                    )
                # consume the two window-DMA semaphores (1 wait each)
                for q in range(2):
                    pe_wait_carrier(xw[:, 0, q, 0:2])
                # output tile: [p, t, m2, j] with out row i = m2*128+p
                ot = otp.tile([128, 16, 2, CROP], f32)
                if c >= 2:
                    # DVE wait carrier for the recycled ot slot: a 2-element
                    # self-copy whose WAR on the finished store-DMA absorbs
                    # that semaphore before the real (PE-waiting) copies
                    nc.vector.tensor_copy(ot[0:1, 0, 0, 0:2], ot[0:1, 0, 1, 0:2])
                for t in range(16):
                    # A^T[w, i] accumulated over h k-tiles
                    psa = psap.tile([128, 2, PAD], f32)
                    for m in range(2):
                        for q in range(2):
                            nc.tensor.matmul(
                                psa[:, m, :],
                                lhsT=xw[:, t, q, m * 128 : (m + 1) * 128],
                                rhs=ryt[:, q, :],
                                start=(q == 0),
                                stop=(q == 1),
                            )
                    at = atp.tile([128, 2, PAD], f32r)
                    nc.vector.tensor_copy(at[:], psa[:].bitcast(f32r))
                    # carrier: PE observes at's DVE tick here, so the
                    # group-start matmul below keeps only its PSUM-drain wait
                    pe_wait_carrier(at[:, 0, 0:2])
                    # out[i, j] accumulated over w k-tiles (out rows 224..255
                    # are the zero pad of A^T's i axis — valid, never stored)
                    pso = psop.tile([128, 2, PAD], f32)
                    for m2 in range(2):
                        for q in range(2):
                            nc.tensor.matmul(
                                pso[:, m2, :],
                                lhsT=at[:, q, m2 * 128 : (m2 + 1) * 128],
                                rhs=rxt[:, q, :],
                                start=(q == 0),
                                stop=(q == 1),
                            )
                    nc.vector.tensor_copy(ot[:, t, :, :], pso[:, :, 0:CROP])
                nc.sync.dma_start(
                    out=out[c, :, 0:128, :].rearrange("t p j -> p t j"),
                    in_=ot[:, :, 0, :],
                )
                nc.sync.dma_start(
                    out=out[c, :, 128:CROP, :].rearrange("t p j -> p t j"),
                    in_=ot[:96, :, 1, :],
                )
    _split_multi_waits(nc)
    return nc


def kernel(x, cam_views, resize, y_offset, x_offset):
    global _PROGRAM, LAST_RESULTS
    from concourse.bass_utils import run_bass_kernel_spmd

    x = np.ascontiguousarray(np.asarray(x), dtype=np.float32)
    cam_views = np.asarray(cam_views)
    resize = np.asarray(resize, dtype=np.float32)
    y_offset = np.asarray(y_offset, dtype=np.float32)
    x_offset = np.asarray(x_offset, dtype=np.float32)

    B = x.shape[0]
    assert x.shape == (8, 3, 16, H, H), x.shape

    # reference's clamp/floor in float32
    r = np.floor(np.clip(resize, np.float32(H), np.float32(RESIZE_MAX)))
    yo = np.floor(np.clip(y_offset, np.float32(0.0), r - np.float32(CROP)))
    xo = np.floor(np.clip(x_offset, np.float32(0.0), r - np.float32(CROP)))

    # per-view interpolation matrices, packed [p, q, PAD] with h = q*128+p
    def pack(m):
        return np.ascontiguousarray(m.reshape(2, 128, PAD).transpose(1, 0, 2))

    ry_v = [pack(_interp_matrix(yo[v], r[v])) for v in range(r.shape[0])]
    rx_v = [pack(_interp_matrix(xo[v], r[v])) for v in range(r.shape[0])]

    if _PROGRAM is None:
        _PROGRAM = _build_program()

    in_maps = []
    for b in range(B):
        v = int(cam_views[b])
        in_maps.append(
            {"xc": np.ascontiguousarray(x[b]), "ry": ry_v[v], "rx": rx_v[v]}
        )

    res = run_bass_kernel_spmd(_PROGRAM, in_maps, list(range(B)), trace=TRACE)
    LAST_RESULTS = res
    return np.stack([res.results[b]["out"] for b in range(B)], axis=0)
